# revision 4
# baseline (speedup 1.0000x reference)
"""GCN2 (GCNII) message-passing kernel for 8 Trainium2 NeuronCores — v3.

The per-layer cost is dominated by dma_gather descriptors (~7ns each on this
part, descriptor-rate-bound, dtype-independent).  v3 therefore:
  - issues one dma_gather per (dst-tile, bank) whose index stream holds the
    real edges first and -1 padding last, with the per-core REAL count
    supplied at runtime via num_idxs_reg (value_load from a count table) —
    padding slots generate NO descriptors (the ucode trims trailing
    negatives), cutting ~17% of gather traffic;
  - keeps the v1 interleaved AGC table layout so the AllGather runs as 4
    chunked collectives that pipeline with the next layer's per-bank gathers;
  - keeps the v2 fp16 datapath (table/gather/indicator/aggregation, fp32
    PSUM), folded normalization (v = 0.9*dinv[src]*dinv[dst], table stores
    plain h), host-folded W' = (1-beta)I + beta*W, identity-matmul residual
    add, and segment-batched indicator builds on the VectorEngine.
"""

import math
import os
import sys

import numpy as np

for _p in ("/opt/trn_rl_repo",):
    if _p not in sys.path and os.path.isdir(_p):
        sys.path.insert(0, _p)

import concourse.bacc as bacc
import concourse.mybir as mybir
import concourse.tile as tile
from concourse.bass_utils import run_bass_kernel_spmd

# ---------------- problem constants (hardcoded per contract) ----------------
N = 100_000
E = 1_600_000
IN_C = 500
HID = 128
OUT_C = 64
L = 8
ALPHA = 0.1
THETA = 0.5

NCORES = 8
NOWN = N // NCORES          # 12500 real nodes per core
NLOC = 12544                # padded to 98 * 128
NT = NLOC // 128            # 98 dst tiles per core
TGS = int(os.environ.get("GCN_TGS", 4))
KIN = 512                   # padded input channels
BANK = 32768                # int16-addressable rows per gather bank
AGC = 4096                  # shard rows per chunked AllGather (AG 0..2)
TROWS = 3 * BANK + NCORES * (NLOC - 3 * AGC)   # 100352 table rows
NBANK = 4

F32 = mybir.dt.float32
F16 = mybir.dt.float16
I16 = mybir.dt.int16
I32 = mybir.dt.int32

_cache = {}

LAST_PERF = {}


def _row_of_node(n):
    """Table row of global node id(s) n (vectorized, AGC-interleaved)."""
    c = n // NOWN
    i = n - c * NOWN
    q = np.minimum(i // AGC, 3)
    tail = NLOC - 3 * AGC  # 256
    return np.where(q < 3, q * BANK + c * AGC + (i - q * AGC),
                    3 * BANK + c * tail + (i - 3 * AGC))


def _preprocess(edge_index):
    """All graph-structure preprocessing on host (numpy)."""
    e0 = edge_index[0].astype(np.int64)
    e1 = edge_index[1].astype(np.int64)
    loop = np.arange(N, dtype=np.int64)
    src = np.concatenate([e0, loop])
    dst = np.concatenate([e1, loop])

    deg = np.bincount(dst, minlength=N).astype(np.float64)
    dinv = np.where(deg > 0, 1.0 / np.sqrt(deg), 0.0).astype(np.float32)

    owner = dst // NOWN
    ldst = dst - owner * NOWN
    t_arr = ldst // 128
    dl_arr = (ldst % 128).astype(np.float32)
    row = _row_of_node(src)
    b_arr = row // BANK
    bidx = (row % BANK).astype(np.int16)
    v_arr = ((1.0 - ALPHA) * dinv[src] * dinv[dst]).astype(np.float32)

    # group id and stable sort
    G = ((owner * NT + t_arr) * NBANK + b_arr).astype(np.int64)
    order = np.argsort(G, kind="stable")
    Gs = G[order]
    counts = np.bincount(Gs, minlength=NCORES * NT * NBANK)
    counts = counts.reshape(NCORES, NT, NBANK)
    C = np.ceil(counts / 128).astype(np.int64).max(axis=0)  # [NT, NBANK]

    # chunk stream plan (identical for all cores)
    tg_tiles = [list(range(g, min(g + TGS, NT))) for g in range(0, NT, TGS)]
    tg_meta = []
    slot0 = np.zeros((NT, NBANK), np.int64)
    ch = 0
    for tiles in tg_tiles:
        tg_ch0 = ch
        banks = []
        for b in range(NBANK):
            b_off = ch - tg_ch0
            ents = []
            for t in tiles:
                slot0[t, b] = ch * 128
                if C[t, b] > 0:
                    ents.append((t, ch - tg_ch0, int(C[t, b])))
                ch += C[t, b]
            banks.append((int(b_off), int(ch - tg_ch0 - b_off), ents))
        tg_meta.append(dict(ch0=int(tg_ch0), nch=int(ch - tg_ch0),
                            banks=banks, tiles=tiles))
    NCHUNK = int(ch)
    NSLOT = NCHUNK * 128
    NCHMAX = max(tg["nch"] for tg in tg_meta)

    # fill per-core flat arrays (vectorized scatter)
    gstart = np.zeros(NCORES * NT * NBANK + 1, np.int64)
    np.cumsum(counts.reshape(-1), out=gstart[1:])
    rank = np.arange(len(Gs)) - gstart[Gs]
    core_of = Gs // (NT * NBANK)
    tb = Gs % (NT * NBANK)
    dest = core_of * NSLOT + slot0.reshape(-1)[tb] + rank

    idx_flat = np.full(NCORES * NSLOT, -1, np.int16)   # pads gather nothing
    dl_flat = np.zeros(NCORES * NSLOT, np.float16)
    v_flat = np.zeros(NCORES * NSLOT, np.float16)
    idx_flat[dest] = bidx[order]
    dl_flat[dest] = dl_arr[order].astype(np.float16)
    v_flat[dest] = v_arr[order].astype(np.float16)

    idx_flat = idx_flat.reshape(NCORES, NSLOT)
    dl_flat = dl_flat.reshape(NCORES, NSLOT)
    v_flat = v_flat.reshape(NCORES, NSLOT)

    # device layouts
    idx_w = np.tile(
        idx_flat.reshape(NCORES, NSLOT // 16, 16).transpose(0, 2, 1), (1, 8, 1)
    ).copy()                                            # [c, 128, NSLOT/16]
    dl_w = dl_flat.reshape(NCORES, NCHUNK, 128).transpose(0, 2, 1).copy()
    v_w = v_flat.reshape(NCORES, NCHUNK, 128).transpose(0, 2, 1).copy()
    cnt_w = counts.transpose(0, 1, 2).reshape(NCORES, 1, NT * NBANK)
    cnt_w = cnt_w.astype(np.int32).copy()               # real edges per (t,b)

    return dict(tg_meta=tg_meta, NCHUNK=NCHUNK, NSLOT=NSLOT, NCHMAX=NCHMAX,
                idx_w=idx_w, dl_w=dl_w, v_w=v_w, cnt_w=cnt_w,
                counts=counts, C=C)


def _build_program(pre, n_layers=L):
    nc = bacc.Bacc("TRN2", target_bir_lowering=False, debug=False,
                   num_devices=NCORES, num_swdge_queues=4)
    tg_meta = pre["tg_meta"]
    gq = [0]  # round-robin SWDGE queue for gathers
    NCHUNK, NSLOT, NCHMAX = pre["NCHUNK"], pre["NSLOT"], pre["NCHMAX"]

    # ---- external inputs ----
    xT_in = nc.dram_tensor("xT", [128, KIN // 128, NLOC], F16, kind="ExternalInput")
    win_in = nc.dram_tensor("win", [128, KIN // 128, HID], F16, kind="ExternalInput")
    bin_in = nc.dram_tensor("bin", [128, 1], F32, kind="ExternalInput")
    wc_in = nc.dram_tensor("wc", [128, L, HID], F16, kind="ExternalInput")
    wout_in = nc.dram_tensor("wout", [128, OUT_C], F16, kind="ExternalInput")
    bout_in = nc.dram_tensor("bout", [128, OUT_C], F16, kind="ExternalInput")
    iota_in = nc.dram_tensor("iota", [128, 1, 128], F16, kind="ExternalInput")
    iota32_in = nc.dram_tensor("iota32", [128, 128], F32, kind="ExternalInput")
    ident_in = nc.dram_tensor("ident", [128, 128], F16, kind="ExternalInput")
    idx_in = nc.dram_tensor("idx", [128, NSLOT // 16], I16, kind="ExternalInput")
    dl_in = nc.dram_tensor("dl", [128, NCHUNK], F16, kind="ExternalInput")
    v_in = nc.dram_tensor("v", [128, NCHUNK], F16, kind="ExternalInput")
    dl32_in = nc.dram_tensor("dl32", [128, NCHUNK], F32, kind="ExternalInput")
    v32_in = nc.dram_tensor("v32", [128, NCHUNK], F32, kind="ExternalInput")
    cnt_in = nc.dram_tensor("cnt", [1, NT * NBANK], I32, kind="ExternalInput")
    out_ext = nc.dram_tensor("out", [NOWN, OUT_C], F32, kind="ExternalOutput")

    rg = [list(range(NCORES))]

    with tile.TileContext(nc, num_cores=NCORES) as tc:
        with (
            tc.tile_pool(name="const", bufs=1) as cpool,
            tc.tile_pool(name="dram", bufs=1, space="DRAM") as dram,
            tc.tile_pool(name="work", bufs=1) as wp,
            tc.tile_pool(name="psum", bufs=1, space="PSUM") as pp,
        ):
            # ---- resident constants ----
            win_sb = cpool.tile([128, KIN // 128, HID], F16)
            bin_sb = cpool.tile([128, 1], F32)
            wc_sb = cpool.tile([128, L, HID], F16)
            wout_sb = cpool.tile([128, OUT_C], F16)
            bout_sb = cpool.tile([128, OUT_C], F16)
            iota_sb = cpool.tile([128, 1, 128], F16)
            iota32_sb = cpool.tile([128, 128], F32)
            ident_sb = cpool.tile([128, 128], F16)
            cnt_sb = cpool.tile([1, NT * NBANK], I32)
            h0sT = cpool.tile([128, NLOC], F16)   # 0.1 * h0^T resident
            for sb_t, ext in ((win_sb, win_in), (bin_sb, bin_in), (wc_sb, wc_in),
                              (wout_sb, wout_in), (bout_sb, bout_in),
                              (iota_sb, iota_in), (iota32_sb, iota32_in),
                              (ident_sb, ident_in),
                              (cnt_sb, cnt_in)):
                nc.sync.dma_start(sb_t[:], ext[:])

            cnt_reg = nc.gpsimd.alloc_register("gcnt")
            NBROWS = [BANK, BANK, BANK, TROWS - 3 * BANK]
            tables = [
                [dram.tile([NBROWS[b], HID], F16, addr_space="Shared",
                           name=f"table{r}_{b}") for b in range(NBANK)]
                for r in range(n_layers)
            ]
            shards = [
                dram.tile([NLOC, HID], F16, name=f"shard{i}") for i in range(2)
            ]

            AG2 = bool(os.environ.get("GCN_AG2"))
            G2 = bool(os.environ.get("GCN_G2"))
            IND2 = bool(os.environ.get("GCN_IND2"))
            MM2 = bool(os.environ.get("GCN_MM2"))

            def ag(shard, tb):
                bounds = [(0, AGC), (AGC, 2 * AGC), (2 * AGC, 3 * AGC),
                          (3 * AGC, NLOC)]
                for b, (r0, r1) in enumerate(bounds):
                    nc.gpsimd.collective_compute(
                        "AllGather", mybir.AluOpType.bypass, replica_groups=rg,
                        ins=[shard[r0:r1, :].opt()],
                        outs=[tb[b][:, :].opt()],
                    )

            # ================= input layer: h0 = relu(x @ W_in + b_in) ======
            shard0 = shards[0]
            for g0 in range(0, NLOC, 512):
                w = min(512, NLOC - g0)
                psin = pp.tile([128, 512], F32, tag="psz", bufs=2)
                for k in range(KIN // 128):
                    xt = wp.tile([128, 512], F16, tag="xt", bufs=3)
                    nc.sync.dma_start(xt[:, :w], xT_in[:, k, g0:g0 + w])
                    nc.tensor.matmul(psin[:, :w], win_sb[:, k, :], xt[:, :w],
                                     start=(k == 0), stop=(k == KIN // 128 - 1))
                h0T = wp.tile([128, 512], F16, tag="h0t", bufs=2)
                nc.scalar.activation(h0T[:, :w], psin[:, :w],
                                     mybir.ActivationFunctionType.Relu,
                                     bias=bin_sb[:, 0:1])
                nc.vector.tensor_scalar(h0sT[:, g0:g0 + w], h0T[:, :w], ALPHA,
                                        None, mybir.AluOpType.mult)
                for j in range(0, w, 128):
                    t = (g0 + j) // 128
                    pst = pp.tile([128, 128], F16, tag="pst", bufs=2)
                    nc.tensor.transpose(pst[:], h0T[:, j:j + 128], ident_sb[:])
                    rows = wp.tile([128, HID], F16, tag="rows", bufs=3)
                    nc.vector.tensor_copy(rows[:], pst[:])
                    nc.sync.dma_start(shard0[t * 128:(t + 1) * 128, :], rows[:])
            ag(shard0, tables[0])

            # ========================= L layers =============================
            for l in range(n_layers):
                t_in = tables[l]
                shard = shards[(l + 1) % 2]
                last = l == n_layers - 1
                for tgi, tg in enumerate(tg_meta):
                    nch = tg["nch"]
                    ch0 = tg["ch0"]
                    idx_t = wp.tile([128, NCHMAX * 8], I16, tag="idxs", bufs=3)
                    nc.sync.dma_start(idx_t[:, :nch * 8],
                                      idx_in[:, ch0 * 8:(ch0 + nch) * 8])
                    dl_t = wp.tile([128, NCHMAX, 1], F16, tag="dlt", bufs=3)
                    nc.sync.dma_start(dl_t[:, :nch, :], dl_in[:, ch0:ch0 + nch])
                    v_t = wp.tile([128, NCHMAX, 1], F16, tag="vt", bufs=3)
                    nc.sync.dma_start(v_t[:, :nch, :], v_in[:, ch0:ch0 + nch])
                    INDC = bool(os.environ.get("GCN_INDC"))
                    if INDC:
                        dl32_t = wp.tile([128, NCHMAX], F32, tag="dl32", bufs=3)
                        nc.sync.dma_start(dl32_t[:, :nch],
                                          dl32_in[:, ch0:ch0 + nch])
                        v32_t = wp.tile([128, NCHMAX], F32, tag="v32", bufs=3)
                        nc.sync.dma_start(v32_t[:, :nch],
                                          v32_in[:, ch0:ch0 + nch])
                    gbuf = wp.tile([128, NCHMAX, HID], F16, tag="g", bufs=2)
                    ind = wp.tile([128, NCHMAX, HID], F16, tag="ind", bufs=2)
                    if l == 0 and tgi < 2:
                        # unwritten pad slots must hold finite values before
                        # the first gathers land (trailing -1 idxs skip DMA)
                        nc.vector.memset(gbuf[:], 0)
                    per_tile = {t: [] for t in tg["tiles"]}
                    for b, (b_off, b_nch, ents) in enumerate(tg["banks"]):
                        if b_nch == 0:
                            continue
                        for (t, pos, cnt) in ents:
                            nc.gpsimd.reg_load(
                                cnt_reg,
                                cnt_sb[0:1, t * NBANK + b:t * NBANK + b + 1])
                            nreg = cnt_reg
                            for _ in range(2 if G2 else 1):
                                nc.gpsimd.dma_gather(
                                    gbuf[:, pos:pos + cnt, :],
                                    t_in[b][:, :],
                                    idx_t[:, pos * 8:(pos + cnt) * 8],
                                    cnt * 128, nreg, HID,
                                    single_packet=bool(
                                        os.environ.get("GCN_SP")),
                                    queue_num=gq[0] % 4,
                                )
                                gq[0] += 1
                            per_tile[t].append((pos, cnt))
                        seg = (slice(None), slice(b_off, b_off + b_nch),
                               slice(None))
                        shp = [128, b_nch, 128]
                        if INDC:
                            # per-chunk fused one-pass build, alternating
                            # between DVE and gpsimd
                            for p in range(b_off, b_off + b_nch):
                                eng = nc.vector
                                eng.tensor_scalar(
                                    ind[:, p, :], iota32_sb[:, :],
                                    dl32_t[:, p:p + 1], v32_t[:, p:p + 1],
                                    mybir.AluOpType.is_equal,
                                    mybir.AluOpType.mult)
                        else:
                            mult_eng = (nc.gpsimd if os.environ.get("GCN_INDP")
                                        else nc.vector)
                            for _ in range(2 if IND2 else 1):
                                nc.vector.scalar_tensor_tensor(
                                    ind[seg],
                                    iota_sb[:, :, :].broadcast_to(shp), 1.0,
                                    dl_t[:, b_off:b_off + b_nch, :]
                                    .broadcast_to(shp),
                                    mybir.AluOpType.mult,
                                    mybir.AluOpType.is_equal)
                                mult_eng.tensor_tensor(
                                    ind[seg], ind[seg],
                                    v_t[:, b_off:b_off + b_nch, :]
                                    .broadcast_to(shp),
                                    mybir.AluOpType.mult)

                    for t in tg["tiles"]:
                        chunks = [p for (pos, cnt) in per_tile[t]
                                  for p in range(pos, pos + cnt)]
                        ps = pp.tile([128, 128], F32, tag="ps", bufs=3)
                        mm_chunks = (chunks + chunks) if MM2 else chunks
                        for k, p in enumerate(mm_chunks):
                            nc.tensor.matmul(ps[:], gbuf[:, p, :], ind[:, p, :],
                                             start=(k == 0), stop=False)
                        # z = agg + 0.1*h0  (identity matmul into same PSUM)
                        nc.tensor.matmul(ps[:], ident_sb[:],
                                         h0sT[:, t * 128:(t + 1) * 128],
                                         start=(len(mm_chunks) == 0), stop=True)
                        zsb = wp.tile([128, 128], F16, tag="z", bufs=3)
                        nc.scalar.copy(zsb[:], ps[:])
                        psz = pp.tile([128, 128], F32, tag="psz", bufs=2)
                        nc.tensor.matmul(psz[:], wc_sb[:, l, :], zsb[:],
                                         start=True, stop=True)
                        hT = wp.tile([128, 128], F16, tag="ht", bufs=3)
                        nc.scalar.activation(
                            hT[:], psz[:], mybir.ActivationFunctionType.Relu)
                        if not last:
                            pst = pp.tile([128, 128], F16, tag="pst", bufs=2)
                            nc.tensor.transpose(pst[:], hT[:], ident_sb[:])
                            rows = wp.tile([128, HID], F16, tag="rows", bufs=3)
                            nc.scalar.copy(rows[:], pst[:])
                            nc.sync.dma_start(
                                shard[t * 128:(t + 1) * 128, :], rows[:])
                        else:
                            # output: out = h @ W_out + b_out
                            pso = pp.tile([64, 128], F32, tag="ps", bufs=3)
                            nc.tensor.matmul(pso[:], wout_sb[:], hT[:],
                                             start=True, stop=True)
                            oT = wp.tile([64, 128], F16, tag="ot", bufs=2)
                            nc.scalar.copy(oT[:], pso[:])
                            psq = pp.tile([128, 64], F16, tag="pst", bufs=2)
                            nc.tensor.transpose(psq[:], oT[:],
                                                ident_sb[:64, :64])
                            ob = wp.tile([128, 64], F32, tag="ob", bufs=3)
                            nc.vector.tensor_tensor(ob[:], psq[:], bout_sb[:],
                                                    mybir.AluOpType.add)
                            r0 = t * 128
                            r1 = min(r0 + 128, NOWN)
                            if r1 > r0:
                                nc.sync.dma_start(out_ext[r0:r1, :],
                                                  ob[:r1 - r0, :])
                    # end tiles
                if not last:
                    ag(shard, tables[l + 1])
                    if AG2:
                        dup = [dram.tile([NBROWS[b], HID], F16,
                                         addr_space="Shared",
                                         name=f"tdup{l}_{b}")
                               for b in range(NBANK)]
                        ag(shard, dup)

    nc.compile()
    return nc


def _host_inputs(inputs, pre):
    x = np.asarray(inputs["x"], np.float32)
    W_in = np.asarray(inputs["W_in"], np.float32)
    b_in = np.asarray(inputs["b_in"], np.float32)
    W_conv = np.asarray(inputs["W_conv"], np.float32)
    W_out = np.asarray(inputs["W_out"], np.float32)
    b_out = np.asarray(inputs["b_out"], np.float32)
    betas = np.array([math.log(THETA / (l + 1) + 1.0) for l in range(L)],
                     np.float32)

    win_w = np.zeros((128, KIN // 128, HID), np.float16)
    for k in range(KIN // 128):
        rows = W_in[k * 128:min((k + 1) * 128, IN_C)]
        win_w[:rows.shape[0], k, :] = rows.astype(np.float16)
    eye = np.eye(HID, dtype=np.float32)
    wc_w = np.stack(
        [(1.0 - betas[l]) * eye + betas[l] * W_conv[l] for l in range(L)],
        axis=1).astype(np.float16)                       # [128, L, 128]
    iota_w = np.tile(np.arange(128, dtype=np.float16)[None, None, :],
                     (128, 1, 1))
    iota32_w = np.tile(np.arange(128, dtype=np.float32)[None, :], (128, 1))
    ident_w = np.eye(128, dtype=np.float16)
    bout_w = np.tile(b_out[None, :], (128, 1)).astype(np.float16)
    bin_w = b_in.reshape(128, 1).astype(np.float32)

    xT_w = np.zeros((NCORES, 128, KIN // 128, NLOC), np.float16)
    xr = x.reshape(NCORES, NOWN, IN_C)
    for k in range(KIN // 128):
        c0, c1 = k * 128, min((k + 1) * 128, IN_C)
        xT_w[:, :c1 - c0, k, :NOWN] = (
            xr[:, :, c0:c1].transpose(0, 2, 1).astype(np.float16))

    maps = []
    for c in range(NCORES):
        maps.append({
            "xT": xT_w[c], "win": win_w, "bin": bin_w, "wc": wc_w,
            "wout": W_out.astype(np.float16), "bout": bout_w,
            "iota": iota_w, "iota32": iota32_w, "ident": ident_w,
            "idx": pre["idx_w"][c],
            "dl": pre["dl_w"][c], "v": pre["v_w"][c],
            "dl32": pre["dl_w"][c].astype(np.float32),
            "v32": pre["v_w"][c].astype(np.float32),
            "cnt": pre["cnt_w"][c],
        })
    return maps


def kernel(**inputs):
    edge_index = np.asarray(inputs["edge_index"])
    key = hash(edge_index.tobytes())
    if key not in _cache:
        pre = _preprocess(edge_index)
        n_layers = int(os.environ.get("GCN_NL", L))
        nc = _build_program(pre, n_layers)
        _cache.clear()
        _cache[key] = (pre, nc)
    pre, nc = _cache[key]

    in_maps = _host_inputs(inputs, pre)
    trace = bool(os.environ.get("GCN_TRACE"))
    res = run_bass_kernel_spmd(nc, in_maps, core_ids=list(range(NCORES)),
                               trace=trace)
    LAST_PERF["exec_time_ns"] = res.exec_time_ns
    LAST_PERF["mean_exec_time_ns"] = res.mean_exec_time_ns
    LAST_PERF["trace"] = (res.instructions_and_trace or (None, None))[1]
    out = np.concatenate([res.results[c]["out"] for c in range(NCORES)], axis=0)
    return out.astype(np.float32)
